# revision 4
# baseline (speedup 1.0000x reference)
"""Trainium2 Bass kernel for nn_Joiner (RNN-T joiner: dense_mlp) — v3.

Reference computation (per batch n):
  enc = encoder_out @ W_enc.T + b_enc           (T=200, J=512)
  dec = decoder_out @ W_dec.T + b_dec           (U=50,  J=512)
  act = tanh(enc[:,None,:] + dec[None,:,:])     (T, U, J)
  out = act @ W_out.T + b_out                   (T, U, V=500)

Sharding: data-parallel over batch N=8 -> one batch element per NeuronCore.

Schedule (per core):
  - bf16 matmul path: host converts inputs/weights to bf16; projections and
    the big output matmul run at 1 cycle/row on the PE.
  - PE warmup matmuls on junk data bridge the input-DMA latency and bring
    the PE out of its low p-state before real work arrives.
  - act stored bf16 in one [128, 4, 10000] SBUF buffer; broadcast add on
    DVE; tanh in-place on the Activation engine with the joint bias
    (b_enc+b_dec) folded into the activation's per-partition bias input.
  - All act blocks are emitted up front (graduated block sizes) so DVE/Act
    stream independently of the PE group loop.
  - logits leave PSUM via f32->bf16 tensor_copy: Pool (GpSimd) handles early
    groups; DVE/Act join for late groups once their own chains have drained.
  - b_out is added on the host after gathering; output is DMA'd bf16.
"""

import numpy as np
import ml_dtypes

N, T, U = 8, 200, 50
E = D = J = 512
V = 500
P = 128
JC = J // P  # 4 j-chunks
POS = T * U  # 10000
# act prep blocks over t (graduated sizes so early groups unblock fast)
T_SIZES = [8, 12, 16, 20, 24, 28, 32, 32, 28]
# pos-tile index from which DVE / Act take over PSUM->SBUF copies
DVE_FROM = 52
ACT_FROM = 52
WARMUP_N = 4
SPLIT_W_DMA = False  # per-jb weight DMAs vs one DMA per weight tensor

BF16 = ml_dtypes.bfloat16

_CACHE = {}


def _split_multi_waits(nc, mybir):
    """Walrus's PE (S3_LW) codegen accepts at most one sync-wait per
    instruction. Tile can emit several. Move every wait of a multi-wait
    instruction onto single-wait NOPs inserted just before it (same engine,
    in-order execution makes this equivalent)."""
    n = 0
    for fn in nc.m.functions:
        for blk in fn.blocks:
            new_insts = []
            for inst in blk.instructions:
                si = inst.sync_info
                if si is not None and len(si.on_wait) > 1:
                    for w in si.on_wait:
                        nop = mybir.InstNoOp(
                            name=f"waitnop-{n}",
                            ins=[],
                            outs=[],
                            sync_info=mybir.SyncInfo(on_wait=[w], on_update=[]),
                            bass_nofuse=True,
                        )
                        n += 1
                        nop.engine = inst.engine
                        new_insts.append(nop)
                    inst.sync_info = mybir.SyncInfo(
                        on_wait=[], on_update=si.on_update
                    )
                new_insts.append(inst)
            blk.instructions[:] = new_insts
    return n


def _build_nc():
    import concourse.bass as bass
    import concourse.tile as tile
    from concourse import mybir

    f32 = mybir.dt.float32
    bf16 = mybir.dt.bfloat16
    AF = mybir.ActivationFunctionType
    ALU = mybir.AluOpType

    nc = bass.Bass("TRN2", target_bir_lowering=False, debug=False, num_devices=8)

    # blob_a: enc (800) + dec (200) + wenc_jb0 (512) + wdec_jb0 (512)
    # blob_b: wenc_jb123 (1536) + wdec_jb123 (1536)
    blob_a_d = nc.dram_tensor("blob_a", [P, 2024], bf16, kind="ExternalInput").ap()
    blob_b_d = nc.dram_tensor("blob_b", [P, 3072], bf16, kind="ExternalInput").ap()
    w_out_d = nc.dram_tensor("wout_p", [P, JC, V], bf16, kind="ExternalInput").ap()
    bsum_d = nc.dram_tensor("bsum", [P, JC], f32, kind="ExternalInput").ap()
    out_d = nc.dram_tensor("out", [POS, V], bf16, kind="ExternalOutput").ap()

    with tile.TileContext(nc) as tc:
        # monotone pseudo-time floors: make the Tile scheduler's per-engine
        # order follow emission order (its own cost model mis-ranks
        # DMA-gated work otherwise)
        _step = [0]

        def tick():
            _step[0] += 1
            tc.tile_set_cur_wait(_step[0] * 1e-4)

        with (
            tc.tile_pool(name="consts", bufs=1) as consts,
            tc.tile_pool(name="psum", bufs=4, space="PSUM") as psum_pool,
        ):
            # ---- load inputs (order = dependency order) ----
            # blob_a carries everything projection jb0 needs; blob_b the
            # remaining per-jb weights; w_out lands right before tile 0.
            blob_a = consts.tile([P, 2024], bf16, tag="blob_a")
            nc.sync.dma_start(blob_a[:], blob_a_d)
            bsum_sb = consts.tile([P, JC], f32, tag="bsum")
            nc.sync.dma_start(bsum_sb[:], bsum_d)
            blob_b = consts.tile([P, 3072], bf16, tag="blob_b")
            nc.sync.dma_start(blob_b[:], blob_b_d)
            w_out_sb = consts.tile([P, JC, V], bf16, tag="w_out")
            nc.sync.dma_start(w_out_sb[:, :2], w_out_d[:, :2])
            nc.sync.dma_start(w_out_sb[:, 2:], w_out_d[:, 2:])

            enc_raw = blob_a[:, 0:800].rearrange("p (c t) -> p c t", c=JC)
            dec_raw = blob_a[:, 800:1000].rearrange("p (c u) -> p c u", c=JC)

            def wenc_jb(jb):
                if jb == 0:
                    return blob_a[:, 1000:1512].rearrange("p (c x) -> p c x", c=JC)
                return blob_b[:, (jb - 1) * 512:jb * 512].rearrange(
                    "p (c x) -> p c x", c=JC
                )

            def wdec_jb(jb):
                if jb == 0:
                    return blob_a[:, 1512:2024].rearrange("p (c x) -> p c x", c=JC)
                return blob_b[:, 1536 + (jb - 1) * 512:1536 + jb * 512].rearrange(
                    "p (c x) -> p c x", c=JC
                )

            enc_sb = consts.tile([P, JC, T], f32, tag="enc_sb")
            dec_sb = consts.tile([P, JC, U], f32, tag="dec_sb")
            act = consts.tile([P, JC, POS], bf16, tag="act")
            outbuf = consts.tile([P, 79, V], bf16, tag="outbuf")

            # ---- PE warmup: junk matmuls with no DMA dependency ----
            warm = consts.tile([P, 512], bf16, tag="warm")
            nc.vector.memset(warm[:], 0.0)
            ps_warm = psum_pool.tile([P, 2, 512], f32, tag="psump")
            for _ in range(WARMUP_N):
                nc.tensor.matmul(
                    ps_warm[:, 0, :], lhsT=warm[:, :P], rhs=warm[:],
                    start=True, stop=True,
                )

            def junk(ps, n):
                # fine-grained filler matmuls: keep the PE busy-chain alive
                # while real dependencies land, so the p-state never resets
                for _ in range(n):
                    nc.tensor.matmul(
                        ps[:, :64], lhsT=warm[:, :P], rhs=warm[:, :64],
                        start=True, stop=True,
                    )

            # ---- projections: enc_sb[j, t], dec_sb[j, u] (J on partitions) ----
            # The first act block reads enc/dec straight from PSUM (the add
            # runs at 1x regardless, and this skips the Pool-copy + sem hop
            # on the critical path); later blocks read the SBUF copies.
            tb0 = T_SIZES[0]

            def add_block(jb, t0, tb, enc_ap, dec_ap, add_eng=None):
                # act[jb, block] = enc (broadcast over u) + biased dec
                # (broadcast over t); the joint bias b_enc+b_dec was folded
                # into dec_sb at projection time, so a single whole-block
                # tanh (no bias) finishes the job.
                seg = act[:, jb, t0 * U:(t0 + tb) * U]
                seg3 = seg.rearrange("p (t u) -> p t u", u=U)
                enc_bc = enc_ap[:, t0:t0 + tb][:, :, None].to_broadcast([P, tb, U])
                dec_bc = dec_ap[:, None, :].to_broadcast([P, tb, U])
                (add_eng or nc.vector).tensor_tensor(
                    out=seg3, in0=enc_bc, in1=dec_bc, op=ALU.add
                )

            def tanh_seg(jlo, jhi, t0, tb):
                seg = act[:, jlo:jhi, t0 * U:(t0 + tb) * U]
                nc.scalar.activation(out=seg, in_=seg, func=AF.Tanh)

            for jb in range(JC):
                tick()
                ps_j = psum_pool.tile([P, 2, 512], f32, tag="psump")
                pe = ps_j[:, 0, :T]
                pd = ps_j[:, 1, :U]
                we = wenc_jb(jb)
                for ec in range(JC):
                    nc.tensor.matmul(
                        pe,
                        lhsT=we[:, ec, :],
                        rhs=enc_raw[:, ec, :],
                        start=(ec == 0),
                        stop=(ec == JC - 1),
                    )
                wd = wdec_jb(jb)
                for ec in range(JC):
                    nc.tensor.matmul(
                        pd,
                        lhsT=wd[:, ec, :],
                        rhs=dec_raw[:, ec, :],
                        start=(ec == 0),
                        stop=(ec == JC - 1),
                    )
                # GPSIMD cannot touch PSUM, so the proj copies ride DVE.
                # dec_sb = dec_proj + (b_enc+b_dec): bias folded here so the
                # block tanh needs no bias. The hw allows only ONE PSUM
                # operand per DVE instruction: enc is read straight from
                # PSUM for block 0, via its SBUF copy for later blocks.
                fl = (4.4 + 0.75 * jb) * 1e-3
                with tc.tile_wait_until(fl):
                    nc.vector.tensor_scalar_add(
                        out=dec_sb[:, jb, :], in0=pd,
                        scalar1=bsum_sb[:, jb:jb + 1],
                    )
                    add_block(jb, 0, tb0, pe, dec_sb[:, jb, :])
                with tc.tile_wait_until(fl + 0.1e-3):
                    # tanh b0 per-jb: latency-critical for tile 0
                    tanh_seg(jb, jb + 1, 0, tb0)
                with tc.tile_wait_until((6.2 + 0.6 * jb) * 1e-3):
                    nc.vector.tensor_copy(out=enc_sb[:, jb, :], in_=pe)

            # ---- static EDF schedule for adds / tanh / logit copies ----
            # Adds are SBUF-only, so they may ride Pool (GPSIMD, no PSUM
            # access allowed there) as well as DVE; the PSUM->SBUF logit
            # copies may ride only DVE/Act. Earliest-deadline-first over a
            # 3-engine model yields engine + pseudo-time floor per item;
            # the Tile scheduler then follows these floors.
            # ---- static schedule for adds / tanh / logit copies ----
            # Per block: jb0/jb1 adds on Pool (SBUF-only, GPSIMD-legal),
            # jb2/jb3 on DVE, one whole-block tanh on Act. Paired-tile logit
            # copies split DVE/Act greedily. Pseudo-time floors steer the
            # Tile scheduler to this interleave.
            items = []  # (order_time, kind, payload)
            cum = tb0 * U
            for k, tb in enumerate(T_SIZES[1:], start=1):
                items.append((6.5 + 4.3e-3 * cum, "block", (k, tb)))
                cum += tb * U
            npairs = (79 + 1) // 2  # 39 full pairs + final single (tile 78)
            for pr in range(npairs):
                ntile = min(2, 79 - pr * 2)
                items.append((9.0 + 0.833 * (pr * 2 + ntile), "copy", (pr, ntile)))
            items.sort(key=lambda it: it[0])
            clock = {"DVE": 6.5, "Pool": 6.5, "Act": 6.5}
            sched = {}
            for rt, kind, pl in items:
                if kind == "block":
                    k, tb = pl
                    w = tb * U
                    ends = {}
                    for jb in range(JC):
                        # Pool takes jb0/jb1 everywhere, plus jb2 of the
                        # two smallest blocks to relieve DVE
                        eng = "Pool" if jb < 2 else "DVE"
                        cost = w * (1.98e-3 if eng == "Pool" else 1.078e-3) + 0.1
                        start = max(clock[eng], rt - 2.0)
                        clock[eng] = start + cost
                        ends[jb] = clock[eng]
                        sched[("add", (k, jb))] = (eng, start)
                    # tanh in two jb-halves for earlier matmul release
                    t1 = max(clock["Act"], ends[0], ends[1])
                    clock["Act"] = t1 + 2 * w * 0.833e-3 + 0.25
                    sched[("tanh", (k, 0))] = ("Act", t1)
                    t2 = max(clock["Act"], ends[2], ends[3])
                    clock["Act"] = t2 + 2 * w * 0.833e-3 + 0.25
                    sched[("tanh", (k, 1))] = ("Act", t2)
                else:
                    pr, ntile = pl
                    w = ntile * 500
                    costs = {"DVE": w * 1.0417e-3 + 0.18,
                             "Act": w * 0.833e-3 + 0.25}
                    eng = min(("DVE", "Act"), key=lambda e: max(clock[e], rt) + costs[e])
                    start = max(clock[eng], rt)
                    clock[eng] = start + costs[eng]
                    sched[("copy", pr)] = (eng, start)

            # ---- act prep, remaining blocks up front ----
            t0 = tb0
            for k, tb in enumerate(T_SIZES[1:], start=1):
                for jb in range(JC):
                    eng, floor = sched[("add", (k, jb))]
                    with tc.tile_wait_until(floor * 1e-3):
                        add_block(
                            jb, t0, tb, enc_sb[:, jb, :], dec_sb[:, jb, :],
                            add_eng=nc.gpsimd if eng == "Pool" else nc.vector,
                        )
                for half in (0, 1):
                    _, tfloor = sched[("tanh", (k, half))]
                    with tc.tile_wait_until(tfloor * 1e-3):
                        tanh_seg(half * 2, half * 2 + 2, t0, tb)
                t0 += tb

            # ---- output matmuls + PSUM->SBUF (bf16) + DMA out ----
            tiles = [(ls, min(P, POS - ls)) for ls in range(0, POS, P)]  # 79
            for pr in range(npairs):
                pair = tiles[pr * 2: pr * 2 + 2]
                ps_t = psum_pool.tile([P, 2, 512], f32, tag="psump")
                for i, (ls, sz) in enumerate(pair):
                    tick()
                    for jb in range(JC):
                        nc.tensor.matmul(
                            ps_t[:sz, i, :V],
                            lhsT=act[:, jb, ls:ls + sz],
                            rhs=w_out_sb[:, jb, :],
                            start=(jb == 0),
                            stop=(jb == JC - 1),
                        )
                eng, floor = sched[("copy", pr)]
                n = len(pair)
                szl = pair[-1][1]
                with tc.tile_wait_until(floor * 1e-3):
                    if n == 2 and szl == P:
                        src = ps_t[:, :2, :V]
                        dst = outbuf[:, pr * 2: pr * 2 + 2, :]
                    else:
                        src = ps_t[:szl, 0, :V]
                        dst = outbuf[:szl, pr * 2, :]
                    if eng == "DVE":
                        nc.vector.tensor_copy(out=dst, in_=src)
                    else:
                        nc.scalar.copy(out=dst, in_=src)
                # one DMA per 2 pairs (4 tiles); small tail transfers
                if pr % 2 == 1 and pr < 38:
                    base = (pr - 1) * 2 * P
                    nc.sync.dma_start(
                        out_d[base: base + 4 * P, :].rearrange(
                            "(g p) v -> p g v", p=P
                        ),
                        outbuf[:, (pr - 1) * 2: (pr - 1) * 2 + 4, :],
                    )
                elif pr == 38:  # tiles 76, 77
                    base = 76 * P
                    nc.sync.dma_start(
                        out_d[base: base + 2 * P, :].rearrange(
                            "(g p) v -> p g v", p=P
                        ),
                        outbuf[:, 76:78, :],
                    )
                elif pr == 39:  # tile 78, 16 rows
                    nc.sync.dma_start(
                        out_d[78 * P: POS, :], outbuf[:POS - 78 * P, 78, :]
                    )
    _split_multi_waits(nc, mybir)
    return nc


def _prep_inputs(encoder_out, decoder_out, W_enc, b_enc, W_dec, b_dec, W_out, b_out):
    def pack_w(w):  # (J_out, K) -> [P, KC, J_out] packing of w.T
        wT = np.ascontiguousarray(np.asarray(w, np.float32).T)
        kc = wT.shape[0] // P
        return np.ascontiguousarray(
            wT.reshape(kc, P, wT.shape[1]).transpose(1, 0, 2).astype(BF16)
        )

    def pack_w4(w):  # (J=512, K=512) -> [p, jb, ec, x] with j=jb*128+x, k=ec*128+p
        wT = np.ascontiguousarray(np.asarray(w, np.float32).T)  # [K, J]
        return np.ascontiguousarray(
            wT.reshape(JC, P, JC, P).transpose(1, 2, 0, 3).astype(BF16)
        )

    w_enc_p = pack_w4(W_enc)  # [128, 4(jb), 4(ec), 128]
    w_dec_p = pack_w4(W_dec)
    w_out_p = pack_w(W_out)   # [128, 4, 500]
    bsum = np.ascontiguousarray(
        (np.asarray(b_enc, np.float32) + np.asarray(b_dec, np.float32))
        .reshape(JC, P).T
    )
    wenc_flat = w_enc_p.reshape(P, JC, JC * P)  # [p, jb, 512]
    wdec_flat = w_dec_p.reshape(P, JC, JC * P)
    blob_b = np.ascontiguousarray(np.concatenate(
        [wenc_flat[:, 1:].reshape(P, -1), wdec_flat[:, 1:].reshape(P, -1)],
        axis=1,
    ))  # [128, 3072]
    in_maps = []
    for n in range(N):
        encT = np.ascontiguousarray(np.asarray(encoder_out[n], np.float32).T)
        decT = np.ascontiguousarray(np.asarray(decoder_out[n], np.float32).T)
        enc_p = encT.reshape(JC, P, T).transpose(1, 0, 2).reshape(P, JC * T)
        dec_p = decT.reshape(JC, P, U).transpose(1, 0, 2).reshape(P, JC * U)
        blob_a = np.ascontiguousarray(np.concatenate(
            [enc_p.astype(BF16), dec_p.astype(BF16),
             wenc_flat[:, 0], wdec_flat[:, 0]],
            axis=1,
        ))  # [128, 2024]
        in_maps.append({
            "blob_a": blob_a,
            "blob_b": blob_b,
            "wout_p": w_out_p,
            "bsum": bsum,
        })
    return in_maps


def get_nc():
    if "nc" not in _CACHE:
        _CACHE["nc"] = _build_nc()
    return _CACHE["nc"]


def run_on_hw(in_maps, trace=False):
    from concourse.bass_utils import run_bass_kernel_spmd

    nc = get_nc()
    return run_bass_kernel_spmd(nc, in_maps, core_ids=list(range(N)), trace=trace)


def kernel(encoder_out, decoder_out, W_enc, b_enc, W_dec, b_dec, W_out, b_out):
    in_maps = _prep_inputs(
        encoder_out, decoder_out, W_enc, b_enc, W_dec, b_dec, W_out, b_out
    )
    res = run_on_hw(in_maps)
    b_out_f = np.asarray(b_out, np.float32)
    out = np.stack(
        [np.asarray(res.results[i]["out"]).astype(np.float32) for i in range(N)],
        axis=0,
    )
    out += b_out_f[None, None, :]
    return out.reshape(N, T, U, V)


# revision 6
# speedup vs baseline: 1.0306x; 1.0306x over previous
"""Trainium2 Bass kernel for nn_Joiner (RNN-T joiner: dense_mlp) — v3.

Reference computation (per batch n):
  enc = encoder_out @ W_enc.T + b_enc           (T=200, J=512)
  dec = decoder_out @ W_dec.T + b_dec           (U=50,  J=512)
  act = tanh(enc[:,None,:] + dec[None,:,:])     (T, U, J)
  out = act @ W_out.T + b_out                   (T, U, V=500)

Sharding: data-parallel over batch N=8 -> one batch element per NeuronCore.

Schedule (per core):
  - bf16 matmul path: host converts inputs/weights to bf16; projections and
    the big output matmul run at 1 cycle/row on the PE (~66us of the ~82us
    total is pure output-matmul time, the compute floor).
  - Inputs arrive as two packed blobs ordered by need (jb0's projection
    weights first) plus w_out split in two, minimizing the serial-DMA time
    before the first tile can run; PE warmup matmuls on junk data cover the
    DMA latency and p-state ramp.
  - act stored bf16 in one [128, 4, 10000] SBUF buffer. The outer-sum add
    runs as broadcast TensorTensor: jb0/jb1 of each t-block on Pool
    (GPSIMD, SBUF-only — it may not touch PSUM), jb2/jb3 on DVE; block 0
    reads enc straight from projection PSUM. tanh runs per jb-half on the
    Activation engine; the joint bias (b_enc+b_dec) is folded into dec_sb
    at projection time.
  - logits leave PSUM as paired-tile f32->bf16 copies split between DVE and
    Act (the only engines allowed to read PSUM) by a static greedy model.
  - A static schedule assigns pseudo-time floors (tc.tile_wait_until) to
    every DVE/Pool/Act item so the Tile scheduler's per-engine order matches
    the intended interleave; its own cost model mis-ranks DMA-gated work.
  - b_out is added on the host after gathering; output is DMA'd bf16 and
    upcast on the host.
"""

import numpy as np
import ml_dtypes

N, T, U = 8, 200, 50
E = D = J = 512
V = 500
P = 128
JC = J // P  # 4 j-chunks
POS = T * U  # 10000
# act prep blocks over t (graduated sizes so early groups unblock fast)
T_SIZES = [8, 12, 16, 20, 24, 28, 32, 32, 28]
# pos-tile index from which DVE / Act take over PSUM->SBUF copies
DVE_FROM = 52
ACT_FROM = 52
WARMUP_N = 4
SPLIT_W_DMA = False  # per-jb weight DMAs vs one DMA per weight tensor

BF16 = ml_dtypes.bfloat16

_CACHE = {}


def _split_multi_waits(nc, mybir):
    """Walrus's PE (S3_LW) codegen accepts at most one sync-wait per
    instruction. Tile can emit several. Move every wait of a multi-wait
    instruction onto single-wait NOPs inserted just before it (same engine,
    in-order execution makes this equivalent)."""
    n = 0
    for fn in nc.m.functions:
        for blk in fn.blocks:
            new_insts = []
            for inst in blk.instructions:
                si = inst.sync_info
                if si is not None and len(si.on_wait) > 1:
                    for w in si.on_wait:
                        nop = mybir.InstNoOp(
                            name=f"waitnop-{n}",
                            ins=[],
                            outs=[],
                            sync_info=mybir.SyncInfo(on_wait=[w], on_update=[]),
                            bass_nofuse=True,
                        )
                        n += 1
                        nop.engine = inst.engine
                        new_insts.append(nop)
                    inst.sync_info = mybir.SyncInfo(
                        on_wait=[], on_update=si.on_update
                    )
                new_insts.append(inst)
            blk.instructions[:] = new_insts
    return n


def _build_nc():
    import concourse.bass as bass
    import concourse.tile as tile
    from concourse import mybir

    f32 = mybir.dt.float32
    bf16 = mybir.dt.bfloat16
    AF = mybir.ActivationFunctionType
    ALU = mybir.AluOpType

    nc = bass.Bass("TRN2", target_bir_lowering=False, debug=False, num_devices=8)

    # blob_a: enc (800) + dec (200) + wenc_jb0 (512) + wdec_jb0 (512)
    # blob_b: wenc_jb123 (1536) + wdec_jb123 (1536)
    blob_a_d = nc.dram_tensor("blob_a", [P, 2024], bf16, kind="ExternalInput").ap()
    blob_b_d = nc.dram_tensor("blob_b", [P, 3072], bf16, kind="ExternalInput").ap()
    w_out_d = nc.dram_tensor("wout_p", [P, JC, V], bf16, kind="ExternalInput").ap()
    bsum_d = nc.dram_tensor("bsum", [P, JC], f32, kind="ExternalInput").ap()
    out_d = nc.dram_tensor("out", [POS, V], bf16, kind="ExternalOutput").ap()

    with tile.TileContext(nc) as tc:
        # monotone pseudo-time floors: make the Tile scheduler's per-engine
        # order follow emission order (its own cost model mis-ranks
        # DMA-gated work otherwise)
        _step = [0]

        def tick():
            _step[0] += 1
            tc.tile_set_cur_wait(_step[0] * 1e-4)

        with (
            tc.tile_pool(name="consts", bufs=1) as consts,
            tc.tile_pool(name="psum", bufs=4, space="PSUM") as psum_pool,
        ):
            # ---- load inputs (order = dependency order) ----
            # blob_a carries everything projection jb0 needs; blob_b the
            # remaining per-jb weights; w_out lands right before tile 0.
            blob_a = consts.tile([P, 2024], bf16, tag="blob_a")
            nc.sync.dma_start(blob_a[:], blob_a_d)
            bsum_sb = consts.tile([P, JC], f32, tag="bsum")
            nc.sync.dma_start(bsum_sb[:], bsum_d)
            blob_b = consts.tile([P, 3072], bf16, tag="blob_b")
            nc.sync.dma_start(blob_b[:, :1024], blob_b_d[:, :1024])
            nc.sync.dma_start(blob_b[:, 1024:], blob_b_d[:, 1024:])
            w_out_sb = consts.tile([P, JC, V], bf16, tag="w_out")
            nc.sync.dma_start(w_out_sb[:, :2], w_out_d[:, :2])
            nc.sync.dma_start(w_out_sb[:, 2:], w_out_d[:, 2:])

            enc_raw = blob_a[:, 0:800].rearrange("p (c t) -> p c t", c=JC)
            dec_raw = blob_a[:, 800:1000].rearrange("p (c u) -> p c u", c=JC)

            def wenc_jb(jb):
                if jb == 0:
                    return blob_a[:, 1000:1512].rearrange("p (c x) -> p c x", c=JC)
                o = (jb - 1) * 1024
                return blob_b[:, o:o + 512].rearrange("p (c x) -> p c x", c=JC)

            def wdec_jb(jb):
                if jb == 0:
                    return blob_a[:, 1512:2024].rearrange("p (c x) -> p c x", c=JC)
                o = (jb - 1) * 1024 + 512
                return blob_b[:, o:o + 512].rearrange("p (c x) -> p c x", c=JC)

            enc_sb = consts.tile([P, JC, T], f32, tag="enc_sb")
            dec_sb = consts.tile([P, JC, U], f32, tag="dec_sb")
            act = consts.tile([P, JC, POS], bf16, tag="act")
            outbuf = consts.tile([P, 79, V], bf16, tag="outbuf")

            # ---- PE warmup: junk matmuls with no DMA dependency ----
            warm = consts.tile([P, 512], bf16, tag="warm")
            nc.vector.memset(warm[:], 0.0)
            ps_warm = psum_pool.tile([P, 2, 512], f32, tag="psump")
            for _ in range(WARMUP_N):
                nc.tensor.matmul(
                    ps_warm[:, 0, :], lhsT=warm[:, :P], rhs=warm[:],
                    start=True, stop=True,
                )

            def junk(ps, n):
                # fine-grained filler matmuls: keep the PE busy-chain alive
                # while real dependencies land, so the p-state never resets
                for _ in range(n):
                    nc.tensor.matmul(
                        ps[:, :64], lhsT=warm[:, :P], rhs=warm[:, :64],
                        start=True, stop=True,
                    )

            # ---- projections: enc_sb[j, t], dec_sb[j, u] (J on partitions) ----
            # The first act block reads enc/dec straight from PSUM (the add
            # runs at 1x regardless, and this skips the Pool-copy + sem hop
            # on the critical path); later blocks read the SBUF copies.
            tb0 = T_SIZES[0]

            def add_block(jb, t0, tb, enc_ap, dec_ap, add_eng=None):
                # act[jb, block] = enc (broadcast over u) + biased dec
                # (broadcast over t); the joint bias b_enc+b_dec was folded
                # into dec_sb at projection time, so a single whole-block
                # tanh (no bias) finishes the job.
                seg = act[:, jb, t0 * U:(t0 + tb) * U]
                seg3 = seg.rearrange("p (t u) -> p t u", u=U)
                enc_bc = enc_ap[:, t0:t0 + tb][:, :, None].to_broadcast([P, tb, U])
                dec_bc = dec_ap[:, None, :].to_broadcast([P, tb, U])
                (add_eng or nc.vector).tensor_tensor(
                    out=seg3, in0=enc_bc, in1=dec_bc, op=ALU.add
                )

            def tanh_seg(jlo, jhi, t0, tb):
                seg = act[:, jlo:jhi, t0 * U:(t0 + tb) * U]
                nc.scalar.activation(out=seg, in_=seg, func=AF.Tanh)

            for jb in range(JC):
                tick()
                ps_j = psum_pool.tile([P, 2, 512], f32, tag="psump")
                pe = ps_j[:, 0, :T]
                pd = ps_j[:, 1, :U]
                we = wenc_jb(jb)
                for ec in range(JC):
                    nc.tensor.matmul(
                        pe,
                        lhsT=we[:, ec, :],
                        rhs=enc_raw[:, ec, :],
                        start=(ec == 0),
                        stop=(ec == JC - 1),
                    )
                wd = wdec_jb(jb)
                for ec in range(JC):
                    nc.tensor.matmul(
                        pd,
                        lhsT=wd[:, ec, :],
                        rhs=dec_raw[:, ec, :],
                        start=(ec == 0),
                        stop=(ec == JC - 1),
                    )
                # GPSIMD cannot touch PSUM, so the proj copies ride DVE.
                # dec_sb = dec_proj + (b_enc+b_dec): bias folded here so the
                # block tanh needs no bias. The hw allows only ONE PSUM
                # operand per DVE instruction: enc is read straight from
                # PSUM for block 0, via its SBUF copy for later blocks.
                fl = (4.4 + 0.75 * jb) * 1e-3
                with tc.tile_wait_until(fl):
                    nc.vector.tensor_scalar_add(
                        out=dec_sb[:, jb, :], in0=pd,
                        scalar1=bsum_sb[:, jb:jb + 1],
                    )
                    add_block(jb, 0, tb0, pe, dec_sb[:, jb, :])
                with tc.tile_wait_until(fl + 0.1e-3):
                    # tanh b0 per-jb: latency-critical for tile 0
                    tanh_seg(jb, jb + 1, 0, tb0)
                with tc.tile_wait_until((6.2 + 0.6 * jb) * 1e-3):
                    nc.vector.tensor_copy(out=enc_sb[:, jb, :], in_=pe)

            # ---- static EDF schedule for adds / tanh / logit copies ----
            # Adds are SBUF-only, so they may ride Pool (GPSIMD, no PSUM
            # access allowed there) as well as DVE; the PSUM->SBUF logit
            # copies may ride only DVE/Act. Earliest-deadline-first over a
            # 3-engine model yields engine + pseudo-time floor per item;
            # the Tile scheduler then follows these floors.
            # ---- static schedule for adds / tanh / logit copies ----
            # Per block: jb0/jb1 adds on Pool (SBUF-only, GPSIMD-legal),
            # jb2/jb3 on DVE, one whole-block tanh on Act. Paired-tile logit
            # copies split DVE/Act greedily. Pseudo-time floors steer the
            # Tile scheduler to this interleave.
            items = []  # (order_time, kind, payload)
            cum = tb0 * U
            for k, tb in enumerate(T_SIZES[1:], start=1):
                items.append((6.5 + 4.3e-3 * cum, "block", (k, tb)))
                cum += tb * U
            npairs = (79 + 1) // 2  # 39 full pairs + final single (tile 78)
            for pr in range(npairs):
                ntile = min(2, 79 - pr * 2)
                items.append((9.0 + 0.833 * (pr * 2 + ntile), "copy", (pr, ntile)))
            items.sort(key=lambda it: it[0])
            clock = {"DVE": 6.5, "Pool": 6.5, "Act": 6.5}
            sched = {}
            for rt, kind, pl in items:
                if kind == "block":
                    k, tb = pl
                    w = tb * U
                    ends = {}
                    for jb in range(JC):
                        # Pool takes jb0/jb1 everywhere, plus jb2 of the
                        # two smallest blocks to relieve DVE
                        eng = "Pool" if (jb < 2 or (jb == 2 and k >= 5)) else "DVE"
                        cost = w * (1.98e-3 if eng == "Pool" else 1.078e-3) + 0.1
                        start = max(clock[eng], rt - 2.0)
                        clock[eng] = start + cost
                        ends[jb] = clock[eng]
                        sched[("add", (k, jb))] = (eng, start)
                    # tanh in two jb-halves for earlier matmul release
                    t1 = max(clock["Act"], ends[0], ends[1])
                    clock["Act"] = t1 + 2 * w * 0.833e-3 + 0.25
                    sched[("tanh", (k, 0))] = ("Act", t1)
                    t2 = max(clock["Act"], ends[2], ends[3])
                    clock["Act"] = t2 + 2 * w * 0.833e-3 + 0.25
                    sched[("tanh", (k, 1))] = ("Act", t2)
                else:
                    pr, ntile = pl
                    w = ntile * 500
                    costs = {"DVE": w * 1.0417e-3 + 0.18,
                             "Act": w * 0.833e-3 + 0.25}
                    eng = min(("DVE", "Act"), key=lambda e: max(clock[e], rt) + costs[e])
                    start = max(clock[eng], rt)
                    clock[eng] = start + costs[eng]
                    sched[("copy", pr)] = (eng, start)

            # ---- act prep, remaining blocks up front ----
            t0 = tb0
            for k, tb in enumerate(T_SIZES[1:], start=1):
                for jb in range(JC):
                    eng, floor = sched[("add", (k, jb))]
                    with tc.tile_wait_until(floor * 1e-3):
                        add_block(
                            jb, t0, tb, enc_sb[:, jb, :], dec_sb[:, jb, :],
                            add_eng=nc.gpsimd if eng == "Pool" else nc.vector,
                        )
                for half in (0, 1):
                    _, tfloor = sched[("tanh", (k, half))]
                    with tc.tile_wait_until(tfloor * 1e-3):
                        tanh_seg(half * 2, half * 2 + 2, t0, tb)
                t0 += tb

            # ---- output matmuls + PSUM->SBUF (bf16) + DMA out ----
            tiles = [(ls, min(P, POS - ls)) for ls in range(0, POS, P)]  # 79
            for pr in range(npairs):
                pair = tiles[pr * 2: pr * 2 + 2]
                ps_t = psum_pool.tile([P, 2, 512], f32, tag="psump")
                for i, (ls, sz) in enumerate(pair):
                    tick()
                    for jb in range(JC):
                        nc.tensor.matmul(
                            ps_t[:sz, i, :V],
                            lhsT=act[:, jb, ls:ls + sz],
                            rhs=w_out_sb[:, jb, :],
                            start=(jb == 0),
                            stop=(jb == JC - 1),
                        )
                eng, floor = sched[("copy", pr)]
                n = len(pair)
                szl = pair[-1][1]
                with tc.tile_wait_until(floor * 1e-3):
                    if n == 2 and szl == P:
                        src = ps_t[:, :2, :V]
                        dst = outbuf[:, pr * 2: pr * 2 + 2, :]
                    else:
                        src = ps_t[:szl, 0, :V]
                        dst = outbuf[:szl, pr * 2, :]
                    if eng == "DVE":
                        nc.vector.tensor_copy(out=dst, in_=src)
                    else:
                        nc.scalar.copy(out=dst, in_=src)
                # one DMA per 2 pairs (4 tiles); small tail transfers
                if pr % 2 == 1 and pr < 38:
                    base = (pr - 1) * 2 * P
                    nc.sync.dma_start(
                        out_d[base: base + 4 * P, :].rearrange(
                            "(g p) v -> p g v", p=P
                        ),
                        outbuf[:, (pr - 1) * 2: (pr - 1) * 2 + 4, :],
                    )
                elif pr == 38:  # tiles 76, 77
                    base = 76 * P
                    nc.sync.dma_start(
                        out_d[base: base + 2 * P, :].rearrange(
                            "(g p) v -> p g v", p=P
                        ),
                        outbuf[:, 76:78, :],
                    )
                elif pr == 39:  # tile 78, 16 rows
                    nc.sync.dma_start(
                        out_d[78 * P: POS, :], outbuf[:POS - 78 * P, 78, :]
                    )
    _split_multi_waits(nc, mybir)
    return nc


def _prep_inputs(encoder_out, decoder_out, W_enc, b_enc, W_dec, b_dec, W_out, b_out):
    def pack_w(w):  # (J_out, K) -> [P, KC, J_out] packing of w.T
        wT = np.ascontiguousarray(np.asarray(w, np.float32).T)
        kc = wT.shape[0] // P
        return np.ascontiguousarray(
            wT.reshape(kc, P, wT.shape[1]).transpose(1, 0, 2).astype(BF16)
        )

    def pack_w4(w):  # (J=512, K=512) -> [p, jb, ec, x] with j=jb*128+x, k=ec*128+p
        wT = np.ascontiguousarray(np.asarray(w, np.float32).T)  # [K, J]
        return np.ascontiguousarray(
            wT.reshape(JC, P, JC, P).transpose(1, 2, 0, 3).astype(BF16)
        )

    w_enc_p = pack_w4(W_enc)  # [128, 4(jb), 4(ec), 128]
    w_dec_p = pack_w4(W_dec)
    w_out_p = pack_w(W_out)   # [128, 4, 500]
    bsum = np.ascontiguousarray(
        (np.asarray(b_enc, np.float32) + np.asarray(b_dec, np.float32))
        .reshape(JC, P).T
    )
    wenc_flat = w_enc_p.reshape(P, JC, JC * P)  # [p, jb, 512]
    wdec_flat = w_dec_p.reshape(P, JC, JC * P)
    blob_b = np.ascontiguousarray(np.concatenate(
        [np.concatenate([wenc_flat[:, k], wdec_flat[:, k]], axis=1)
         for k in (1, 2, 3)],
        axis=1,
    ))  # [128, 3072]: [wenc1, wdec1, wenc2, wdec2, wenc3, wdec3]
    in_maps = []
    for n in range(N):
        encT = np.ascontiguousarray(np.asarray(encoder_out[n], np.float32).T)
        decT = np.ascontiguousarray(np.asarray(decoder_out[n], np.float32).T)
        enc_p = encT.reshape(JC, P, T).transpose(1, 0, 2).reshape(P, JC * T)
        dec_p = decT.reshape(JC, P, U).transpose(1, 0, 2).reshape(P, JC * U)
        blob_a = np.ascontiguousarray(np.concatenate(
            [enc_p.astype(BF16), dec_p.astype(BF16),
             wenc_flat[:, 0], wdec_flat[:, 0]],
            axis=1,
        ))  # [128, 2024]
        in_maps.append({
            "blob_a": blob_a,
            "blob_b": blob_b,
            "wout_p": w_out_p,
            "bsum": bsum,
        })
    return in_maps


def get_nc():
    if "nc" not in _CACHE:
        _CACHE["nc"] = _build_nc()
    return _CACHE["nc"]


def run_on_hw(in_maps, trace=False):
    from concourse.bass_utils import run_bass_kernel_spmd

    nc = get_nc()
    return run_bass_kernel_spmd(nc, in_maps, core_ids=list(range(N)), trace=trace)


def kernel(encoder_out, decoder_out, W_enc, b_enc, W_dec, b_dec, W_out, b_out):
    in_maps = _prep_inputs(
        encoder_out, decoder_out, W_enc, b_enc, W_dec, b_dec, W_out, b_out
    )
    res = run_on_hw(in_maps)
    b_out_f = np.asarray(b_out, np.float32)
    out = np.stack(
        [np.asarray(res.results[i]["out"]).astype(np.float32) for i in range(N)],
        axis=0,
    )
    out += b_out_f[None, None, :]
    return out.reshape(N, T, U, V)


# revision 7
# speedup vs baseline: 1.0310x; 1.0003x over previous
"""Trainium2 Bass kernel for nn_Joiner (RNN-T joiner: dense_mlp) — v3.

Reference computation (per batch n):
  enc = encoder_out @ W_enc.T + b_enc           (T=200, J=512)
  dec = decoder_out @ W_dec.T + b_dec           (U=50,  J=512)
  act = tanh(enc[:,None,:] + dec[None,:,:])     (T, U, J)
  out = act @ W_out.T + b_out                   (T, U, V=500)

Sharding: data-parallel over batch N=8 -> one batch element per NeuronCore.

Schedule (per core):
  - bf16 matmul path: host converts inputs/weights to bf16; projections and
    the big output matmul run at 1 cycle/row on the PE (~66us of the ~82us
    total is pure output-matmul time, the compute floor).
  - Inputs arrive as two packed blobs ordered by need (jb0's projection
    weights first) plus w_out split in two, minimizing the serial-DMA time
    before the first tile can run; PE warmup matmuls on junk data cover the
    DMA latency and p-state ramp.
  - act stored bf16 in one [128, 4, 10000] SBUF buffer. The outer-sum add
    runs as broadcast TensorTensor: jb0/jb1 of each t-block on Pool
    (GPSIMD, SBUF-only — it may not touch PSUM), jb2/jb3 on DVE; block 0
    reads enc straight from projection PSUM. tanh runs per jb-half on the
    Activation engine; the joint bias (b_enc+b_dec) is folded into dec_sb
    at projection time.
  - logits leave PSUM as paired-tile f32->bf16 copies split between DVE and
    Act (the only engines allowed to read PSUM) by a static greedy model.
  - A static schedule assigns pseudo-time floors (tc.tile_wait_until) to
    every DVE/Pool/Act item so the Tile scheduler's per-engine order matches
    the intended interleave; its own cost model mis-ranks DMA-gated work.
  - b_out is added on the host after gathering; output is DMA'd bf16 and
    upcast on the host.
"""

import numpy as np
import ml_dtypes

N, T, U = 8, 200, 50
E = D = J = 512
V = 500
P = 128
JC = J // P  # 4 j-chunks
POS = T * U  # 10000
# act prep blocks over t (graduated sizes so early groups unblock fast)
T_SIZES = [8, 12, 16, 20, 24, 28, 32, 32, 28]
# pos-tile index from which DVE / Act take over PSUM->SBUF copies
DVE_FROM = 52
ACT_FROM = 52
WARMUP_N = 4
SPLIT_W_DMA = False  # per-jb weight DMAs vs one DMA per weight tensor

BF16 = ml_dtypes.bfloat16

_CACHE = {}


def _split_multi_waits(nc, mybir):
    """Walrus's PE (S3_LW) codegen accepts at most one sync-wait per
    instruction. Tile can emit several. Move every wait of a multi-wait
    instruction onto single-wait NOPs inserted just before it (same engine,
    in-order execution makes this equivalent)."""
    n = 0
    for fn in nc.m.functions:
        for blk in fn.blocks:
            new_insts = []
            for inst in blk.instructions:
                si = inst.sync_info
                if si is not None and len(si.on_wait) > 1:
                    for w in si.on_wait:
                        nop = mybir.InstNoOp(
                            name=f"waitnop-{n}",
                            ins=[],
                            outs=[],
                            sync_info=mybir.SyncInfo(on_wait=[w], on_update=[]),
                            bass_nofuse=True,
                        )
                        n += 1
                        nop.engine = inst.engine
                        new_insts.append(nop)
                    inst.sync_info = mybir.SyncInfo(
                        on_wait=[], on_update=si.on_update
                    )
                new_insts.append(inst)
            blk.instructions[:] = new_insts
    return n


def _build_nc():
    import concourse.bass as bass
    import concourse.tile as tile
    from concourse import mybir

    f32 = mybir.dt.float32
    bf16 = mybir.dt.bfloat16
    AF = mybir.ActivationFunctionType
    ALU = mybir.AluOpType

    nc = bass.Bass("TRN2", target_bir_lowering=False, debug=False, num_devices=8)

    # blob_a: enc (800) + dec (200) + wenc_jb0 (512) + wdec_jb0 (512)
    # blob_b: wenc_jb123 (1536) + wdec_jb123 (1536)
    blob_a_d = nc.dram_tensor("blob_a", [P, 2024], bf16, kind="ExternalInput").ap()
    blob_b_d = nc.dram_tensor("blob_b", [P, 3072], bf16, kind="ExternalInput").ap()
    w_out_d = nc.dram_tensor("wout_p", [P, JC, V], bf16, kind="ExternalInput").ap()
    bsum_d = nc.dram_tensor("bsum", [P, JC], f32, kind="ExternalInput").ap()
    out_d = nc.dram_tensor("out", [POS, V], bf16, kind="ExternalOutput").ap()

    with tile.TileContext(nc) as tc:
        # monotone pseudo-time floors: make the Tile scheduler's per-engine
        # order follow emission order (its own cost model mis-ranks
        # DMA-gated work otherwise)
        _step = [0]

        def tick():
            _step[0] += 1
            tc.tile_set_cur_wait(_step[0] * 1e-4)

        with (
            tc.tile_pool(name="consts", bufs=1) as consts,
            tc.tile_pool(name="psum", bufs=4, space="PSUM") as psum_pool,
        ):
            # ---- load inputs (order = dependency order) ----
            # blob_a carries everything projection jb0 needs; blob_b the
            # remaining per-jb weights; w_out lands right before tile 0.
            blob_a = consts.tile([P, 2024], bf16, tag="blob_a")
            nc.sync.dma_start(blob_a[:], blob_a_d)
            bsum_sb = consts.tile([P, JC], f32, tag="bsum")
            nc.sync.dma_start(bsum_sb[:], bsum_d)
            blob_b = consts.tile([P, 3072], bf16, tag="blob_b")
            nc.sync.dma_start(blob_b[:, :1024], blob_b_d[:, :1024])
            nc.sync.dma_start(blob_b[:, 1024:], blob_b_d[:, 1024:])
            w_out_sb = consts.tile([P, JC, V], bf16, tag="w_out")
            nc.sync.dma_start(w_out_sb[:, :2], w_out_d[:, :2])
            nc.sync.dma_start(w_out_sb[:, 2:], w_out_d[:, 2:])

            enc_raw = blob_a[:, 0:800].rearrange("p (c t) -> p c t", c=JC)
            dec_raw = blob_a[:, 800:1000].rearrange("p (c u) -> p c u", c=JC)

            def wenc_jb(jb):
                if jb == 0:
                    return blob_a[:, 1000:1512].rearrange("p (c x) -> p c x", c=JC)
                o = (jb - 1) * 1024
                return blob_b[:, o:o + 512].rearrange("p (c x) -> p c x", c=JC)

            def wdec_jb(jb):
                if jb == 0:
                    return blob_a[:, 1512:2024].rearrange("p (c x) -> p c x", c=JC)
                o = (jb - 1) * 1024 + 512
                return blob_b[:, o:o + 512].rearrange("p (c x) -> p c x", c=JC)

            enc_sb = consts.tile([P, JC, T], f32, tag="enc_sb")
            dec_sb = consts.tile([P, JC, U], f32, tag="dec_sb")
            act = consts.tile([P, JC, POS], bf16, tag="act")
            outbuf = consts.tile([P, 79, V], bf16, tag="outbuf")

            # ---- PE warmup: junk matmuls with no DMA dependency ----
            warm = consts.tile([P, 512], bf16, tag="warm")
            nc.vector.memset(warm[:], 0.0)
            ps_warm = psum_pool.tile([P, 2, 512], f32, tag="psump")
            for _ in range(WARMUP_N):
                nc.tensor.matmul(
                    ps_warm[:, 0, :], lhsT=warm[:, :P], rhs=warm[:],
                    start=True, stop=True,
                )

            def junk(ps, n):
                # fine-grained filler matmuls: keep the PE busy-chain alive
                # while real dependencies land, so the p-state never resets
                for _ in range(n):
                    nc.tensor.matmul(
                        ps[:, :64], lhsT=warm[:, :P], rhs=warm[:, :64],
                        start=True, stop=True,
                    )

            # ---- projections: enc_sb[j, t], dec_sb[j, u] (J on partitions) ----
            # The first act block reads enc/dec straight from PSUM (the add
            # runs at 1x regardless, and this skips the Pool-copy + sem hop
            # on the critical path); later blocks read the SBUF copies.
            tb0 = T_SIZES[0]

            def add_block(jb, t0, tb, enc_ap, dec_ap, add_eng=None):
                # act[jb, block] = enc (broadcast over u) + biased dec
                # (broadcast over t); the joint bias b_enc+b_dec was folded
                # into dec_sb at projection time, so a single whole-block
                # tanh (no bias) finishes the job.
                seg = act[:, jb, t0 * U:(t0 + tb) * U]
                seg3 = seg.rearrange("p (t u) -> p t u", u=U)
                enc_bc = enc_ap[:, t0:t0 + tb][:, :, None].to_broadcast([P, tb, U])
                dec_bc = dec_ap[:, None, :].to_broadcast([P, tb, U])
                (add_eng or nc.vector).tensor_tensor(
                    out=seg3, in0=enc_bc, in1=dec_bc, op=ALU.add
                )

            def tanh_seg(jlo, jhi, t0, tb):
                seg = act[:, jlo:jhi, t0 * U:(t0 + tb) * U]
                nc.scalar.activation(out=seg, in_=seg, func=AF.Tanh)

            for jb in range(JC):
                tick()
                ps_j = psum_pool.tile([P, 2, 512], f32, tag="psump")
                pe = ps_j[:, 0, :T]
                pd = ps_j[:, 1, :U]
                we = wenc_jb(jb)
                for ec in range(JC):
                    nc.tensor.matmul(
                        pe,
                        lhsT=we[:, ec, :],
                        rhs=enc_raw[:, ec, :],
                        start=(ec == 0),
                        stop=(ec == JC - 1),
                    )
                wd = wdec_jb(jb)
                for ec in range(JC):
                    nc.tensor.matmul(
                        pd,
                        lhsT=wd[:, ec, :],
                        rhs=dec_raw[:, ec, :],
                        start=(ec == 0),
                        stop=(ec == JC - 1),
                    )
                # GPSIMD cannot touch PSUM, so the proj copies ride DVE.
                # dec_sb = dec_proj + (b_enc+b_dec): bias folded here so the
                # block tanh needs no bias. The hw allows only ONE PSUM
                # operand per DVE instruction: enc is read straight from
                # PSUM for block 0, via its SBUF copy for later blocks.
                fl = (4.4 + 0.75 * jb) * 1e-3
                with tc.tile_wait_until(fl):
                    nc.vector.tensor_scalar_add(
                        out=dec_sb[:, jb, :], in0=pd,
                        scalar1=bsum_sb[:, jb:jb + 1],
                    )
                    add_block(jb, 0, tb0, pe, dec_sb[:, jb, :])
                with tc.tile_wait_until(fl + 0.1e-3):
                    # tanh b0 per-jb: latency-critical for tile 0
                    tanh_seg(jb, jb + 1, 0, tb0)
                with tc.tile_wait_until((6.2 + 0.6 * jb) * 1e-3):
                    nc.vector.tensor_copy(out=enc_sb[:, jb, :], in_=pe)

            # ---- static EDF schedule for adds / tanh / logit copies ----
            # Adds are SBUF-only, so they may ride Pool (GPSIMD, no PSUM
            # access allowed there) as well as DVE; the PSUM->SBUF logit
            # copies may ride only DVE/Act. Earliest-deadline-first over a
            # 3-engine model yields engine + pseudo-time floor per item;
            # the Tile scheduler then follows these floors.
            # ---- static schedule for adds / tanh / logit copies ----
            # Per block: jb0/jb1 adds on Pool (SBUF-only, GPSIMD-legal),
            # jb2/jb3 on DVE, one whole-block tanh on Act. Paired-tile logit
            # copies split DVE/Act greedily. Pseudo-time floors steer the
            # Tile scheduler to this interleave.
            items = []  # (order_time, kind, payload)
            cum = tb0 * U
            for k, tb in enumerate(T_SIZES[1:], start=1):
                items.append((6.5 + 4.3e-3 * cum, "block", (k, tb)))
                cum += tb * U
            npairs = (79 + 1) // 2  # 39 full pairs + final single (tile 78)
            for pr in range(npairs):
                ntile = min(2, 79 - pr * 2)
                items.append((8.4 + 0.833 * (pr * 2 + ntile), "copy", (pr, ntile)))
            items.sort(key=lambda it: it[0])
            clock = {"DVE": 6.5, "Pool": 6.5, "Act": 6.5}
            sched = {}
            for rt, kind, pl in items:
                if kind == "block":
                    k, tb = pl
                    w = tb * U
                    ends = {}
                    for jb in range(JC):
                        # Pool takes jb0/jb1 everywhere, plus jb2 of the
                        # two smallest blocks to relieve DVE
                        eng = "Pool" if (jb < 2 or (jb == 2 and k >= 5)) else "DVE"
                        cost = w * (1.98e-3 if eng == "Pool" else 1.078e-3) + 0.1
                        start = max(clock[eng], rt - 2.0)
                        clock[eng] = start + cost
                        ends[jb] = clock[eng]
                        sched[("add", (k, jb))] = (eng, start)
                    # tanh in two jb-halves for earlier matmul release
                    t1 = max(clock["Act"], ends[0], ends[1])
                    clock["Act"] = t1 + 2 * w * 0.833e-3 + 0.25
                    sched[("tanh", (k, 0))] = ("Act", t1)
                    t2 = max(clock["Act"], ends[2], ends[3])
                    clock["Act"] = t2 + 2 * w * 0.833e-3 + 0.25
                    sched[("tanh", (k, 1))] = ("Act", t2)
                else:
                    pr, ntile = pl
                    w = ntile * 500
                    costs = {"DVE": w * 1.0417e-3 + 0.18,
                             "Act": w * 0.833e-3 + 0.25}
                    eng = min(("DVE", "Act"), key=lambda e: max(clock[e], rt) + costs[e])
                    start = max(clock[eng], rt)
                    clock[eng] = start + costs[eng]
                    sched[("copy", pr)] = (eng, start)

            # ---- act prep, remaining blocks up front ----
            t0 = tb0
            for k, tb in enumerate(T_SIZES[1:], start=1):
                for jb in range(JC):
                    eng, floor = sched[("add", (k, jb))]
                    with tc.tile_wait_until(floor * 1e-3):
                        add_block(
                            jb, t0, tb, enc_sb[:, jb, :], dec_sb[:, jb, :],
                            add_eng=nc.gpsimd if eng == "Pool" else nc.vector,
                        )
                for half in (0, 1):
                    _, tfloor = sched[("tanh", (k, half))]
                    with tc.tile_wait_until(tfloor * 1e-3):
                        tanh_seg(half * 2, half * 2 + 2, t0, tb)
                t0 += tb

            # ---- output matmuls + PSUM->SBUF (bf16) + DMA out ----
            tiles = [(ls, min(P, POS - ls)) for ls in range(0, POS, P)]  # 79
            for pr in range(npairs):
                pair = tiles[pr * 2: pr * 2 + 2]
                ps_t = psum_pool.tile([P, 2, 512], f32, tag="psump")
                for i, (ls, sz) in enumerate(pair):
                    tick()
                    for jb in range(JC):
                        nc.tensor.matmul(
                            ps_t[:sz, i, :V],
                            lhsT=act[:, jb, ls:ls + sz],
                            rhs=w_out_sb[:, jb, :],
                            start=(jb == 0),
                            stop=(jb == JC - 1),
                        )
                eng, floor = sched[("copy", pr)]
                n = len(pair)
                szl = pair[-1][1]
                with tc.tile_wait_until(floor * 1e-3):
                    if n == 2 and szl == P:
                        src = ps_t[:, :2, :V]
                        dst = outbuf[:, pr * 2: pr * 2 + 2, :]
                    else:
                        src = ps_t[:szl, 0, :V]
                        dst = outbuf[:szl, pr * 2, :]
                    if eng == "DVE":
                        nc.vector.tensor_copy(out=dst, in_=src)
                    else:
                        nc.scalar.copy(out=dst, in_=src)
                # one DMA per 2 pairs (4 tiles); small tail transfers
                if pr % 2 == 1 and pr < 38:
                    base = (pr - 1) * 2 * P
                    nc.sync.dma_start(
                        out_d[base: base + 4 * P, :].rearrange(
                            "(g p) v -> p g v", p=P
                        ),
                        outbuf[:, (pr - 1) * 2: (pr - 1) * 2 + 4, :],
                    )
                elif pr == 38:  # tiles 76, 77
                    base = 76 * P
                    nc.sync.dma_start(
                        out_d[base: base + 2 * P, :].rearrange(
                            "(g p) v -> p g v", p=P
                        ),
                        outbuf[:, 76:78, :],
                    )
                elif pr == 39:  # tile 78, 16 rows
                    nc.sync.dma_start(
                        out_d[78 * P: POS, :], outbuf[:POS - 78 * P, 78, :]
                    )
    _split_multi_waits(nc, mybir)
    return nc


def _prep_inputs(encoder_out, decoder_out, W_enc, b_enc, W_dec, b_dec, W_out, b_out):
    def pack_w(w):  # (J_out, K) -> [P, KC, J_out] packing of w.T
        wT = np.ascontiguousarray(np.asarray(w, np.float32).T)
        kc = wT.shape[0] // P
        return np.ascontiguousarray(
            wT.reshape(kc, P, wT.shape[1]).transpose(1, 0, 2).astype(BF16)
        )

    def pack_w4(w):  # (J=512, K=512) -> [p, jb, ec, x] with j=jb*128+x, k=ec*128+p
        wT = np.ascontiguousarray(np.asarray(w, np.float32).T)  # [K, J]
        return np.ascontiguousarray(
            wT.reshape(JC, P, JC, P).transpose(1, 2, 0, 3).astype(BF16)
        )

    w_enc_p = pack_w4(W_enc)  # [128, 4(jb), 4(ec), 128]
    w_dec_p = pack_w4(W_dec)
    w_out_p = pack_w(W_out)   # [128, 4, 500]
    bsum = np.ascontiguousarray(
        (np.asarray(b_enc, np.float32) + np.asarray(b_dec, np.float32))
        .reshape(JC, P).T
    )
    wenc_flat = w_enc_p.reshape(P, JC, JC * P)  # [p, jb, 512]
    wdec_flat = w_dec_p.reshape(P, JC, JC * P)
    blob_b = np.ascontiguousarray(np.concatenate(
        [np.concatenate([wenc_flat[:, k], wdec_flat[:, k]], axis=1)
         for k in (1, 2, 3)],
        axis=1,
    ))  # [128, 3072]: [wenc1, wdec1, wenc2, wdec2, wenc3, wdec3]
    in_maps = []
    for n in range(N):
        encT = np.ascontiguousarray(np.asarray(encoder_out[n], np.float32).T)
        decT = np.ascontiguousarray(np.asarray(decoder_out[n], np.float32).T)
        enc_p = encT.reshape(JC, P, T).transpose(1, 0, 2).reshape(P, JC * T)
        dec_p = decT.reshape(JC, P, U).transpose(1, 0, 2).reshape(P, JC * U)
        blob_a = np.ascontiguousarray(np.concatenate(
            [enc_p.astype(BF16), dec_p.astype(BF16),
             wenc_flat[:, 0], wdec_flat[:, 0]],
            axis=1,
        ))  # [128, 2024]
        in_maps.append({
            "blob_a": blob_a,
            "blob_b": blob_b,
            "wout_p": w_out_p,
            "bsum": bsum,
        })
    return in_maps


def get_nc():
    if "nc" not in _CACHE:
        _CACHE["nc"] = _build_nc()
    return _CACHE["nc"]


def run_on_hw(in_maps, trace=False):
    from concourse.bass_utils import run_bass_kernel_spmd

    nc = get_nc()
    return run_bass_kernel_spmd(nc, in_maps, core_ids=list(range(N)), trace=trace)


def kernel(encoder_out, decoder_out, W_enc, b_enc, W_dec, b_dec, W_out, b_out):
    in_maps = _prep_inputs(
        encoder_out, decoder_out, W_enc, b_enc, W_dec, b_dec, W_out, b_out
    )
    res = run_on_hw(in_maps)
    b_out_f = np.asarray(b_out, np.float32)
    out = np.stack(
        [np.asarray(res.results[i]["out"]).astype(np.float32) for i in range(N)],
        axis=0,
    )
    out += b_out_f[None, None, :]
    return out.reshape(N, T, U, V)
